# revision 11
# baseline (speedup 1.0000x reference)
"""Trainium2 Bass kernel for nn_ConvTemporalGraphical (gnn_message_passing).

Reference computation (see problem):
    A_full[o,k,v,w] = A[k,v,w] * importance[o,k,v,w]          (O,K,V,V)
    y[n,o,t,w]      = sum_{k,v} x[n,0,t,v] * A_full[o,k,v,w]  (N,O,T,V)
    returns (y, A_full)

Only channel 0 of x is used. The k-sum factors out of the x contraction:
    y[n,o,t,w] = sum_v x[n,0,t,v] * B[o,v,w],  B = sum_k A_full[o,k,:,:]
so the device work is a skinny matmul: (N*T, 25) @ (25, 200) per batch shard.

Sharding: data-parallel over batch N across 8 cores (8 batches/core ->
4096 tokens/core). B / importance are tiny and replicated.

Device layout per core:
  - x^T is packed on host into 3 row-groups at partition bases 0/32/64
    (the only legal engine AP bases), 11/11/10 token-tiles per group:
    xp[32*j + v, t'] = x^T[v, tok_off_j + t'].  DMAs are 96 partitions
    wide with multi-KB contiguous runs.
  - B is padded to (25, 256) free (moving free dim 256 => float32r
    streams 1 row/cycle) and replicated at partition bases 0/32/64.
  - 32 matmuls: lhsT = xp[32j:32j+25, 128-token slice] (stationary),
    rhs = bpad[32j:32j+25, :256] (moving), out = PSUM (128, 256) fp32.
  - PSUM -> SBUF copies (DVE/ACT split), staged 4 token-tiles per ybuf,
    8 output DMAs of (128, 4, 200).
  - A_full (an output) is computed on device as impT * at8 in
    [v, (o,k,w)] layout and DMA'd out; host restores the (O,K,V,V) order.
"""

import numpy as np

N, C, T, V = 64, 64, 512, 25
O, K = 8, 3
NCORES = 8
NLOC = N // NCORES          # 8 batches per core
TOK = NLOC * T              # 4096 tokens per core
OW = O * V                  # 200
NTILE = TOK // 128          # 32 token tiles

# 3 row-groups at partition bases 0/32/64: (ntiles) per group
GRP_NTILES = [11, 11, 10]
GRP_TOFF = [0, 11, 22]      # first global tile of each group
XCOLS = 128 * max(GRP_NTILES)      # 1408
XCHUNK_COLS = [512, 512, 384]      # column chunks (128-aligned)

# fp32 matmul is exact (4 cycles/moving-row); float32r is ~TF32 precision
# (~3e-4 rel err on this problem) but streams 1 cycle/row at N>=256.
USE_F32R = False
NPAD = 256 if USE_F32R else OW

_CACHE = {}


def _tile_to_group(gt):
    for j in range(3):
        if gt < GRP_TOFF[j] + GRP_NTILES[j]:
            return j, gt - GRP_TOFF[j]
    raise ValueError(gt)


def _build_nc():
    import concourse.bacc as bacc
    import concourse.mybir as mybir
    import concourse.tile as tile

    f32 = mybir.dt.float32
    mmdt = mybir.dt.float32r if USE_F32R else f32

    nc = bacc.Bacc("TRN2", target_bir_lowering=False, debug=False,
                   enable_asserts=False)

    xp_d = nc.dram_tensor("xp", [96, XCOLS], mmdt, kind="ExternalInput")
    bpad_d = nc.dram_tensor("bpad", [96, NPAD], mmdt, kind="ExternalInput")
    impt_d = nc.dram_tensor("impt", [V, O * K * V], f32, kind="ExternalInput")
    at8_d = nc.dram_tensor("at8", [V, O * K * V], f32, kind="ExternalInput")
    y_d = nc.dram_tensor("y", [NTILE, 128, OW], f32, kind="ExternalOutput")
    afullt_d = nc.dram_tensor("afullt", [V, O * K * V], f32, kind="ExternalOutput")

    with tile.TileContext(nc) as tc:
        with (
            tc.tile_pool(name="const", bufs=1) as cpool,
            tc.tile_pool(name="ybuf", bufs=6) as ypool,
            tc.tile_pool(name="psum", bufs=8, space="PSUM") as pspool,
        ):
            # B weights (host-prepped, replicated at bases 0/32/64):
            # issue on the ACT HWDGE ring, concurrent with x on the SP ring.
            bp = cpool.tile([96, NPAD], mmdt)
            nc.scalar.dma_start(bp[:], bpad_d[:])

            # x^T in 3 column chunks on the SP ring.
            xch = []
            cb = 0
            for cidx, cw in enumerate(XCHUNK_COLS):
                t = cpool.tile([96, cw], mmdt, tag=f"xch{cidx}")
                nc.sync.dma_start(t[:], xp_d[:, cb:cb + cw])
                xch.append((cb, t))
                cb += cw

            # Tiny A_full inputs on gpsimd (SWDGE) to keep HWDGE rings clear.
            im = cpool.tile([V, O * K * V], f32)
            nc.gpsimd.dma_start(im[:], impt_d[:])
            a8 = cpool.tile([V, O * K * V], f32)
            nc.gpsimd.dma_start(a8[:], at8_d[:])

            # Main loop, round-robin across the 3 row-groups so consecutive
            # matmuls land on independent 32-row PE tiles (they overlap).
            # Each token tile flushes to DRAM immediately after its copy;
            # DMA issuance alternates between the SP and ACT HWDGE rings.
            order = [(j, lt) for lt in range(max(GRP_NTILES))
                     for j in range(3) if lt < GRP_NTILES[j]]
            for (j, lt) in order:
                gt = GRP_TOFF[j] + lt
                col = 128 * lt
                chunk = min(col // 512, 2)
                cb, xt = xch[chunk]
                ps = pspool.tile([128, NPAD], f32, tag="ps")
                lhsT = xt[32 * j:32 * j + V, col - cb:col - cb + 128]
                rhs = bp[32 * j:32 * j + V, :]
                nc.tensor.matmul(ps[:], lhsT, rhs)
                yt = ypool.tile([128, OW], f32, name="yt", tag="yt")
                if j < 2:
                    nc.vector.tensor_copy(yt[:], ps[:, 0:OW])
                    nc.sync.dma_start(y_d[gt], yt[:])
                else:
                    nc.scalar.copy(yt[:], ps[:, 0:OW])
                    nc.scalar.dma_start(y_d[gt], yt[:])

            # A_full output: prod[v, (o,k,w)] = impT * at8 — fully off the
            # critical path (DVE op + SWDGE store, emitted last).
            pr = cpool.tile([V, O * K * V], f32)
            nc.vector.tensor_mul(pr[:], im[:], a8[:])
            nc.gpsimd.dma_start(afullt_d[:], pr[:])

    nc.compile()
    return nc


def _get_nc():
    if "nc" not in _CACHE:
        _CACHE["nc"] = _build_nc()
    return _CACHE["nc"]


def _host_prep(x, A, importance):
    """Returns in_maps (list of 8 dicts keyed by DRAM tensor name)."""
    x = np.asarray(x, dtype=np.float32)
    A = np.asarray(A, dtype=np.float32)
    importance = np.asarray(importance, dtype=np.float32)

    x0 = np.ascontiguousarray(x[:, 0, :, :])            # (N, T, V)

    a_full = A[None, :, :, :] * importance              # (O, K, V, V)
    B = (a_full[:, 0] + a_full[:, 1]) + a_full[:, 2]    # (O, V, V)
    bmat = B.transpose(1, 0, 2).reshape(V, OW)          # [v, o*V + w]
    bpad = np.zeros((96, NPAD), np.float32)
    for j in range(3):
        bpad[32 * j:32 * j + V, :OW] = bmat

    impt = np.ascontiguousarray(
        importance.transpose(2, 0, 1, 3).reshape(V, O * K * V))
    at8 = np.ascontiguousarray(
        np.tile(A.transpose(1, 0, 2).reshape(V, K * V), (1, O)))

    in_maps = []
    for c in range(NCORES):
        xt = x0[c * NLOC:(c + 1) * NLOC].reshape(TOK, V).T   # (V, TOK)
        xpk = np.zeros((96, XCOLS), np.float32)
        for j in range(3):
            ncols = 128 * GRP_NTILES[j]
            t0 = 128 * GRP_TOFF[j]
            xpk[32 * j:32 * j + V, :ncols] = xt[:, t0:t0 + ncols]
        in_maps.append({
            "xp": xpk,
            "bpad": bpad,
            "impt": impt,
            "at8": at8,
        })
    return in_maps


def _gather(results):
    y = np.empty((N, O, T, V), np.float32)
    for c in range(NCORES):
        yc = np.asarray(results[c]["y"]).reshape(NLOC, T, O, V)
        y[c * NLOC:(c + 1) * NLOC] = yc.transpose(0, 2, 1, 3)
    aft = np.asarray(results[0]["afullt"])               # (V, O*K*V)
    a_full = np.ascontiguousarray(
        aft.reshape(V, O, K, V).transpose(1, 2, 0, 3))   # (O, K, V, V)
    return y, a_full


def kernel(x, A, importance):
    from concourse.bass_utils import run_bass_kernel_spmd

    nc = _get_nc()
    in_maps = _host_prep(x, A, importance)
    res = run_bass_kernel_spmd(nc, in_maps, core_ids=list(range(NCORES)))
    _CACHE["last_results"] = res
    return _gather(res.results)


# revision 14
# speedup vs baseline: 1.1149x; 1.1149x over previous
"""Trainium2 Bass kernel for nn_ConvTemporalGraphical (gnn_message_passing).

Reference computation (see problem):
    A_full[o,k,v,w] = A[k,v,w] * importance[o,k,v,w]          (O,K,V,V)
    y[n,o,t,w]      = sum_{k,v} x[n,0,t,v] * A_full[o,k,v,w]  (N,O,T,V)
    returns (y, A_full)

Only channel 0 of x is used. The k-sum factors out of the x contraction:
    y[n,o,t,w] = sum_v x[n,0,t,v] * B[o,v,w],  B = sum_k A_full[o,k,:,:]
so the device work is a skinny matmul: (N*T, 25) @ (25, 200) per batch shard.

Sharding: data-parallel over batch N across 8 cores (8 batches/core ->
4096 tokens/core). B / importance are tiny and replicated.

Device layout per core:
  - x^T is packed on host into 3 row-groups at partition bases 0/32/64
    (the only legal engine AP bases), 11/11/10 token-tiles per group:
    xp[32*j + v, t'] = x^T[v, tok_off_j + t'].  DMAs are 96 partitions
    wide with multi-KB contiguous runs.
  - B is padded to (25, 256) free (moving free dim 256 => float32r
    streams 1 row/cycle) and replicated at partition bases 0/32/64.
  - 32 matmuls: lhsT = xp[32j:32j+25, 128-token slice] (stationary),
    rhs = bpad[32j:32j+25, :256] (moving), out = PSUM (128, 256) fp32.
  - PSUM -> SBUF copies (DVE/ACT split), staged 4 token-tiles per ybuf,
    8 output DMAs of (128, 4, 200).
  - A_full (an output) is computed on device as impT * at8 in
    [v, (o,k,w)] layout and DMA'd out; host restores the (O,K,V,V) order.
"""

import numpy as np

N, C, T, V = 64, 64, 512, 25
O, K = 8, 3
NCORES = 8
NLOC = N // NCORES          # 8 batches per core
TOK = NLOC * T              # 4096 tokens per core
OW = O * V                  # 200
NTILE = TOK // 128          # 32 token tiles

# 3 row-groups at partition bases 0/32/64: (ntiles) per group
GRP_NTILES = [11, 11, 10]
GRP_TOFF = [0, 11, 22]      # first global tile of each group
XCOLS = 128 * max(GRP_NTILES)      # 1408
XCHUNK_COLS = [512, 512, 384]      # column chunks (128-aligned)

# fp32 matmul is exact (4 cycles/moving-row); float32r is ~TF32 precision
# (~3e-4 rel err on this problem) but streams 1 cycle/row at N>=256.
USE_F32R = False
NPAD = 256 if USE_F32R else OW

_CACHE = {}


def _tile_to_group(gt):
    for j in range(3):
        if gt < GRP_TOFF[j] + GRP_NTILES[j]:
            return j, gt - GRP_TOFF[j]
    raise ValueError(gt)


def _build_nc():
    import concourse.bacc as bacc
    import concourse.mybir as mybir
    import concourse.tile as tile

    f32 = mybir.dt.float32
    mmdt = mybir.dt.float32r if USE_F32R else f32

    nc = bacc.Bacc("TRN2", target_bir_lowering=False, debug=False,
                   enable_asserts=False)

    xp_d = nc.dram_tensor("xp", [96, XCOLS], mmdt, kind="ExternalInput")
    bpad_d = nc.dram_tensor("bpad", [96, NPAD], mmdt, kind="ExternalInput")
    impt_d = nc.dram_tensor("impt", [V, O * K * V], f32, kind="ExternalInput")
    at8_d = nc.dram_tensor("at8", [V, O * K * V], f32, kind="ExternalInput")
    # y is stored PARTITION-MAJOR: [p, round(lt), group(j), w-col]. Each
    # partition's data for a flush of consecutive rounds is contiguous in
    # DRAM, so a 2-round flush is 128 descriptors of 4800B (cheap HWDGE
    # issue + efficient SDMA runs). Host restores token-major order.
    y_d = nc.dram_tensor("y", [128, max(GRP_NTILES), 3, OW], f32,
                         kind="ExternalOutput")
    afullt_d = nc.dram_tensor("afullt", [V, O * K * V], f32, kind="ExternalOutput")

    with tile.TileContext(nc) as tc:
        with (
            tc.tile_pool(name="const", bufs=1) as cpool,
            tc.tile_pool(name="ybuf", bufs=6) as ypool,
            tc.tile_pool(name="psum", bufs=8, space="PSUM") as pspool,
        ):
            # B weights (host-prepped, replicated at bases 0/32/64):
            # issue on the ACT HWDGE ring, concurrent with x on the SP ring.
            bp = cpool.tile([96, NPAD], mmdt)
            nc.scalar.dma_start(bp[:], bpad_d[:])

            # x^T in 3 column chunks on the SP ring.
            xch = []
            cb = 0
            for cidx, cw in enumerate(XCHUNK_COLS):
                t = cpool.tile([96, cw], mmdt, tag=f"xch{cidx}")
                nc.sync.dma_start(t[:], xp_d[:, cb:cb + cw])
                xch.append((cb, t))
                cb += cw

            # Tiny A_full inputs on gpsimd (SWDGE) to keep HWDGE rings clear.
            im = cpool.tile([V, O * K * V], f32)
            nc.gpsimd.dma_start(im[:], impt_d[:])
            a8 = cpool.tile([V, O * K * V], f32)
            nc.gpsimd.dma_start(a8[:], at8_d[:])

            # Main loop, round-robin across the 3 row-groups so consecutive
            # matmuls land on independent 32-row PE tiles (they overlap).
            # Rounds are staged in pairs in ybuf and flushed with one
            # 128-descriptor DMA per pair on the SP ring.
            yb = None
            for lt in range(max(GRP_NTILES)):
                if lt % 2 == 0:
                    yb = ypool.tile([128, 2, 3, OW], f32, name="yb", tag="yb")
                for j in range(3):
                    if lt >= GRP_NTILES[j]:
                        continue
                    col = 128 * lt
                    chunk = min(col // 512, 2)
                    cb, xt = xch[chunk]
                    ps = pspool.tile([128, NPAD], f32, tag="ps")
                    lhsT = xt[32 * j:32 * j + V, col - cb:col - cb + 128]
                    rhs = bp[32 * j:32 * j + V, :]
                    nc.tensor.matmul(ps[:], lhsT, rhs)
                    if j < 2:
                        nc.vector.tensor_copy(yb[:, lt % 2, j, :], ps[:, 0:OW])
                    else:
                        nc.scalar.copy(yb[:, lt % 2, j, :], ps[:, 0:OW])
                if lt % 2 == 1:
                    nc.sync.dma_start(y_d[:, lt - 1:lt + 1, :, :], yb[:])
            # last round (lt=10) has groups 0,1 only
            nc.sync.dma_start(y_d[:, 10:11, 0:2, :], yb[:, 0:1, 0:2, :])

            # A_full output: prod[v, (o,k,w)] = impT * at8 — fully off the
            # critical path (DVE op + SWDGE store, emitted last).
            pr = cpool.tile([V, O * K * V], f32)
            nc.vector.tensor_mul(pr[:], im[:], a8[:])
            nc.gpsimd.dma_start(afullt_d[:], pr[:])

    nc.compile()
    return nc


def _get_nc():
    if "nc" not in _CACHE:
        _CACHE["nc"] = _build_nc()
    return _CACHE["nc"]


def _host_prep(x, A, importance):
    """Returns in_maps (list of 8 dicts keyed by DRAM tensor name)."""
    x = np.asarray(x, dtype=np.float32)
    A = np.asarray(A, dtype=np.float32)
    importance = np.asarray(importance, dtype=np.float32)

    x0 = np.ascontiguousarray(x[:, 0, :, :])            # (N, T, V)

    a_full = A[None, :, :, :] * importance              # (O, K, V, V)
    B = (a_full[:, 0] + a_full[:, 1]) + a_full[:, 2]    # (O, V, V)
    bmat = B.transpose(1, 0, 2).reshape(V, OW)          # [v, o*V + w]
    bpad = np.zeros((96, NPAD), np.float32)
    for j in range(3):
        bpad[32 * j:32 * j + V, :OW] = bmat

    impt = np.ascontiguousarray(
        importance.transpose(2, 0, 1, 3).reshape(V, O * K * V))
    at8 = np.ascontiguousarray(
        np.tile(A.transpose(1, 0, 2).reshape(V, K * V), (1, O)))

    in_maps = []
    for c in range(NCORES):
        xt = x0[c * NLOC:(c + 1) * NLOC].reshape(TOK, V).T   # (V, TOK)
        xpk = np.zeros((96, XCOLS), np.float32)
        for j in range(3):
            ncols = 128 * GRP_NTILES[j]
            t0 = 128 * GRP_TOFF[j]
            xpk[32 * j:32 * j + V, :ncols] = xt[:, t0:t0 + ncols]
        in_maps.append({
            "xp": xpk,
            "bpad": bpad,
            "impt": impt,
            "at8": at8,
        })
    return in_maps


def _gather(results):
    y = np.empty((N, O, T, V), np.float32)
    for c in range(NCORES):
        yr = np.asarray(results[c]["y"])        # (128, 11, 3, OW) p-major
        yt = np.transpose(yr, (1, 2, 0, 3))     # (lt, j, p, OW)
        tok = np.empty((NTILE, 128, OW), np.float32)
        for j in range(3):
            nt = GRP_NTILES[j]
            tok[GRP_TOFF[j]:GRP_TOFF[j] + nt] = yt[:nt, j]
        yc = tok.reshape(NLOC, T, O, V)
        y[c * NLOC:(c + 1) * NLOC] = yc.transpose(0, 2, 1, 3)
    aft = np.asarray(results[0]["afullt"])               # (V, O*K*V)
    a_full = np.ascontiguousarray(
        aft.reshape(V, O, K, V).transpose(1, 2, 0, 3))   # (O, K, V, V)
    return y, a_full


def kernel(x, A, importance):
    from concourse.bass_utils import run_bass_kernel_spmd

    nc = _get_nc()
    in_maps = _host_prep(x, A, importance)
    res = run_bass_kernel_spmd(nc, in_maps, core_ids=list(range(NCORES)))
    _CACHE["last_results"] = res
    return _gather(res.results)


# revision 17
# speedup vs baseline: 1.1396x; 1.0222x over previous
"""Trainium2 Bass kernel for nn_ConvTemporalGraphical (gnn_message_passing).

Reference computation (see problem):
    A_full[o,k,v,w] = A[k,v,w] * importance[o,k,v,w]          (O,K,V,V)
    y[n,o,t,w]      = sum_{k,v} x[n,0,t,v] * A_full[o,k,v,w]  (N,O,T,V)
    returns (y, A_full)

Only channel 0 of x is used. The k-sum factors out of the x contraction:
    y[n,o,t,w] = sum_v x[n,0,t,v] * B[o,v,w],  B = sum_k A_full[o,k,:,:]
so the device work is a skinny matmul: (N*T, 25) @ (25, 200) per batch shard.

Sharding: data-parallel over batch N across 8 cores (8 batches/core ->
4096 tokens/core). B / importance are tiny and replicated.

Device layout per core:
  - x^T is packed on host into 3 row-groups at partition bases 0/32/64
    (the only legal engine AP bases), 11/11/10 token-tiles per group:
    xp[32*j + v, t'] = x^T[v, tok_off_j + t'].  DMAs are 96 partitions
    wide with multi-KB contiguous runs.
  - B is padded to (25, 256) free (moving free dim 256 => float32r
    streams 1 row/cycle) and replicated at partition bases 0/32/64.
  - 32 matmuls: lhsT = xp[32j:32j+25, 128-token slice] (stationary),
    rhs = bpad[32j:32j+25, :256] (moving), out = PSUM (128, 256) fp32.
  - PSUM -> SBUF copies (DVE/ACT split), staged 4 token-tiles per ybuf,
    8 output DMAs of (128, 4, 200).
  - A_full (an output) is computed on device as impT * at8 in
    [v, (o,k,w)] layout and DMA'd out; host restores the (O,K,V,V) order.
"""

import numpy as np

N, C, T, V = 64, 64, 512, 25
O, K = 8, 3
NCORES = 8
NLOC = N // NCORES          # 8 batches per core
TOK = NLOC * T              # 4096 tokens per core
OW = O * V                  # 200
NTILE = TOK // 128          # 32 token tiles

# 3 row-groups at partition bases 0/32/64: (ntiles) per group
GRP_NTILES = [11, 11, 10]
GRP_TOFF = [0, 11, 22]      # first global tile of each group
XCOLS = 128 * max(GRP_NTILES)      # 1408
XCHUNK_COLS = [512, 512, 384]      # column chunks (128-aligned)

# fp32 matmul is exact (4 cycles/moving-row); float32r is ~TF32 precision
# (~3e-4 rel err on this problem) but streams 1 cycle/row at N>=256.
USE_F32R = False
NPAD = 256 if USE_F32R else OW

# raw-Bass pipeline (manual semaphores) avoids the Tile framework's
# preamble/tail barriers (~10us on a ~20us kernel)
USE_RAW = True

_CACHE = {}


def _tile_to_group(gt):
    for j in range(3):
        if gt < GRP_TOFF[j] + GRP_NTILES[j]:
            return j, gt - GRP_TOFF[j]
    raise ValueError(gt)


def _build_nc():
    import concourse.bacc as bacc
    import concourse.mybir as mybir
    import concourse.tile as tile

    f32 = mybir.dt.float32
    mmdt = mybir.dt.float32r if USE_F32R else f32

    nc = bacc.Bacc("TRN2", target_bir_lowering=False, debug=False,
                   enable_asserts=False)

    xp_d = nc.dram_tensor("xp", [96, XCOLS], mmdt, kind="ExternalInput")
    bpad_d = nc.dram_tensor("bpad", [96, NPAD], mmdt, kind="ExternalInput")
    impt_d = nc.dram_tensor("impt", [V, O * K * V], f32, kind="ExternalInput")
    at8_d = nc.dram_tensor("at8", [V, O * K * V], f32, kind="ExternalInput")
    # y is stored PARTITION-MAJOR: [p, round(lt), group(j), w-col]. Each
    # partition's data for a flush of consecutive rounds is contiguous in
    # DRAM, so a 2-round flush is 128 descriptors of 4800B (cheap HWDGE
    # issue + efficient SDMA runs). Host restores token-major order.
    y_d = nc.dram_tensor("y", [128, max(GRP_NTILES), 3, OW], f32,
                         kind="ExternalOutput")
    afullt_d = nc.dram_tensor("afullt", [V, O * K * V], f32, kind="ExternalOutput")

    with tile.TileContext(nc) as tc:
        with (
            tc.tile_pool(name="const", bufs=1) as cpool,
            tc.tile_pool(name="ybuf", bufs=6) as ypool,
            tc.tile_pool(name="psum", bufs=8, space="PSUM") as pspool,
        ):
            # B weights (host-prepped, replicated at bases 0/32/64):
            # issue on the ACT HWDGE ring, concurrent with x on the SP ring.
            bp = cpool.tile([96, NPAD], mmdt)
            nc.scalar.dma_start(bp[:], bpad_d[:])

            # x^T in 3 column chunks on the SP ring.
            xch = []
            cb = 0
            for cidx, cw in enumerate(XCHUNK_COLS):
                t = cpool.tile([96, cw], mmdt, tag=f"xch{cidx}")
                nc.sync.dma_start(t[:], xp_d[:, cb:cb + cw])
                xch.append((cb, t))
                cb += cw

            # Tiny A_full inputs on gpsimd (SWDGE) to keep HWDGE rings clear.
            im = cpool.tile([V, O * K * V], f32)
            nc.gpsimd.dma_start(im[:], impt_d[:])
            a8 = cpool.tile([V, O * K * V], f32)
            nc.gpsimd.dma_start(a8[:], at8_d[:])

            # Main loop, round-robin across the 3 row-groups so consecutive
            # matmuls land on independent 32-row PE tiles (they overlap).
            # Rounds are staged in pairs in ybuf and flushed with one
            # 128-descriptor DMA per pair on the SP ring.
            yb = None
            for lt in range(max(GRP_NTILES)):
                if lt % 2 == 0:
                    yb = ypool.tile([128, 2, 3, OW], f32, name="yb", tag="yb")
                for j in range(3):
                    if lt >= GRP_NTILES[j]:
                        continue
                    col = 128 * lt
                    chunk = min(col // 512, 2)
                    cb, xt = xch[chunk]
                    ps = pspool.tile([128, NPAD], f32, tag="ps")
                    lhsT = xt[32 * j:32 * j + V, col - cb:col - cb + 128]
                    rhs = bp[32 * j:32 * j + V, :]
                    nc.tensor.matmul(ps[:], lhsT, rhs)
                    if j < 2:
                        nc.vector.tensor_copy(yb[:, lt % 2, j, :], ps[:, 0:OW])
                    else:
                        nc.scalar.copy(yb[:, lt % 2, j, :], ps[:, 0:OW])
                if lt % 2 == 1:
                    nc.sync.dma_start(y_d[:, lt - 1:lt + 1, :, :], yb[:])
            # last round (lt=10) has groups 0,1 only
            nc.sync.dma_start(y_d[:, 10:11, 0:2, :], yb[:, 0:1, 0:2, :])

            # A_full output: prod[v, (o,k,w)] = impT * at8 — fully off the
            # critical path (DVE op + SWDGE store, emitted last).
            pr = cpool.tile([V, O * K * V], f32)
            nc.vector.tensor_mul(pr[:], im[:], a8[:])
            nc.gpsimd.dma_start(afullt_d[:], pr[:])

    nc.compile()
    return nc


def _mm_order():
    """MM index i -> (lt, j); round-robin j inner so consecutive matmuls hit
    independent 32-row PE tiles."""
    out = []
    for lt in range(max(GRP_NTILES)):
        for j in range(3):
            if lt < GRP_NTILES[j]:
                out.append((lt, j))
    return out


def _build_nc_raw():
    """Hand-scheduled pipeline: no TileContext, manual semaphores.

    Engine programs:
      sync   : x chunk loads, then one y flush per round-pair
      scalar : bpad load, then the j==2 PSUM->SBUF copies
      vector : the j<2 copies, then the A_full product
      tensor : 32 fp32 matmuls (waits: inputs, PSUM slot recycled)
      gpsimd : impT/at8 loads, A_full store
    """
    import contextlib

    import concourse.bass as bass
    import concourse.mybir as mybir

    f32 = mybir.dt.float32
    mmdt = mybir.dt.float32r if USE_F32R else f32

    nc = bass.Bass("TRN2", target_bir_lowering=False, debug=False,
                   enable_asserts=False)

    xp_d = nc.dram_tensor("xp", [96, XCOLS], mmdt, kind="ExternalInput")
    bpad_d = nc.dram_tensor("bpad", [96, NPAD], mmdt, kind="ExternalInput")
    impt_d = nc.dram_tensor("impt", [V, O * K * V], f32, kind="ExternalInput")
    at8_d = nc.dram_tensor("at8", [V, O * K * V], f32, kind="ExternalInput")
    y_d = nc.dram_tensor("y", [128, max(GRP_NTILES), 3, OW], f32,
                         kind="ExternalOutput")
    afullt_d = nc.dram_tensor("afullt", [V, O * K * V], f32, kind="ExternalOutput")

    order = _mm_order()
    nmm = len(order)                       # 32
    chunk_of = lambda lt: min(128 * lt // 512, 2)
    # copy engine per MM: DVE for j<2, ACT for j==2
    cp_eng = ["dve" if j < 2 else "act" for (lt, j) in order]
    # cumulative per-engine copy counts after MM i completes its copy
    dve_cum, act_cum = [], []
    dv = ac = 0
    for e in cp_eng:
        if e == "dve":
            dv += 1
        else:
            ac += 1
        dve_cum.append(dv)
        act_cum.append(ac)
    npairs = (max(GRP_NTILES) + 1) // 2    # 6 (last is the lone lt=10 round)
    # per-pair cumulative copy counts (y flush p waits for these)
    pair_last_mm = [max(i for i, (lt, j) in enumerate(order) if lt // 2 == p)
                    for p in range(npairs)]

    with contextlib.ExitStack() as ctx:
        sb = lambda shape, dt_, name: ctx.enter_context(
            nc.sbuf_tensor(name, shape, dt_))
        xch = [sb([96, XCHUNK_COLS[c]], mmdt, f"xch{c}") for c in range(3)]
        bp = sb([96, NPAD], mmdt, "bp")
        ybuf = [sb([128, 2 * 3 * OW], f32, f"ybuf{b}") for b in range(3)]
        im = sb([V, O * K * V], f32, "im")
        a8 = sb([V, O * K * V], f32, "a8")
        pr = sb([V, O * K * V], f32, "pr")
        psum = [ctx.enter_context(
            nc.psum_tensor(f"ps{s}", [128, 512], f32)) for s in range(8)]

        sem_x = [ctx.enter_context(nc.semaphore(f"sem_x{c}")) for c in range(3)]
        sem_b = ctx.enter_context(nc.semaphore("sem_b"))
        sem_g = ctx.enter_context(nc.semaphore("sem_g"))
        sem_yb = [ctx.enter_context(nc.semaphore(f"sem_yb{b}")) for b in range(3)]
        mm_sem = ctx.enter_context(nc.semaphore("mm_sem"))
        cp_dve = ctx.enter_context(nc.semaphore("cp_dve"))
        cp_act = ctx.enter_context(nc.semaphore("cp_act"))
        mul_sem = ctx.enter_context(nc.semaphore("mul_sem"))

        block = ctx.enter_context(nc.Block())

        @block.sync
        def _(sync):
            cb = 0
            for c in range(3):
                sync.dma_start(
                    xch[c][:], xp_d[:, cb:cb + XCHUNK_COLS[c]]
                ).then_inc(sem_x[c], 16)
                cb += XCHUNK_COLS[c]
            for p in range(npairs):
                i_last = pair_last_mm[p]
                sync.wait_ge(cp_dve, dve_cum[i_last])
                if act_cum[i_last]:
                    sync.wait_ge(cp_act, act_cum[i_last])
                if p < npairs - 1:
                    sync.dma_start(
                        y_d[:, 2 * p:2 * p + 2, :, :],
                        ybuf[p % 3][:].rearrange(
                            "p (r j c) -> p r j c", r=2, j=3),
                    ).then_inc(sem_yb[p % 3], 16)
                else:  # lone lt=10 round: groups 0,1 only
                    sync.dma_start(
                        y_d[:, 10:11, 0:2, :],
                        ybuf[p % 3][:, 0:2 * OW].rearrange(
                            "p (r j c) -> p r j c", r=1, j=2),
                    ).then_inc(sem_yb[p % 3], 16)
            for b in range(3):
                sync.wait_ge(sem_yb[b], 32)

        @block.tensor
        def _(tensor):
            emitted_x = 0
            for i, (lt, j) in enumerate(order):
                ch = chunk_of(lt)
                if i == 0:
                    tensor.wait_ge(sem_b, 16)
                if ch + 1 > emitted_x:
                    tensor.wait_ge(sem_x[ch], 16)
                    emitted_x = ch + 1
                if i >= 8:
                    ip = i - 8
                    if cp_eng[ip] == "dve":
                        tensor.wait_ge(cp_dve, dve_cum[ip])
                    else:
                        tensor.wait_ge(cp_act, act_cum[ip])
                col = 128 * lt
                cb = [0, 512, 1024][ch]
                tensor.matmul(
                    psum[i % 8][:, 0:NPAD],
                    xch[ch][32 * j:32 * j + V, col - cb:col - cb + 128],
                    bp[32 * j:32 * j + V, :],
                ).then_inc(mm_sem)

        @block.vector
        def _(vector):
            seen_pair = -1
            for i, (lt, j) in enumerate(order):
                if cp_eng[i] != "dve":
                    continue
                p = lt // 2
                if p >= 3 and p != seen_pair:
                    vector.wait_ge(sem_yb[p % 3], 16)
                seen_pair = max(seen_pair, p)
                vector.wait_ge(mm_sem, i + 1)
                vector.tensor_copy(
                    ybuf[p % 3][:, ((lt % 2) * 3 + j) * OW:
                                ((lt % 2) * 3 + j + 1) * OW],
                    psum[i % 8][:, 0:OW],
                ).then_inc(cp_dve)
            vector.wait_ge(sem_g, 32)
            vector.tensor_mul(pr[:], im[:], a8[:]).then_inc(mul_sem)

        @block.scalar
        def _(scalar):
            scalar.dma_start(bp[:], bpad_d[:]).then_inc(sem_b, 16)
            seen_pair = -1
            for i, (lt, j) in enumerate(order):
                if cp_eng[i] != "act":
                    continue
                p = lt // 2
                if p >= 3 and p != seen_pair:
                    scalar.wait_ge(sem_yb[p % 3], 16)
                seen_pair = max(seen_pair, p)
                scalar.wait_ge(mm_sem, i + 1)
                scalar.copy(
                    ybuf[p % 3][:, ((lt % 2) * 3 + j) * OW:
                                ((lt % 2) * 3 + j + 1) * OW],
                    psum[i % 8][:, 0:OW],
                ).then_inc(cp_act)

        @block.gpsimd
        def _(gpsimd):
            gpsimd.dma_start(im[:], impt_d[:]).then_inc(sem_g, 16)
            gpsimd.dma_start(a8[:], at8_d[:]).then_inc(sem_g, 16)
            gpsimd.wait_ge(mul_sem, 1)
            gpsimd.dma_start(afullt_d[:], pr[:]).then_inc(sem_g, 16)
            gpsimd.wait_ge(sem_g, 48)

    return nc


def _get_nc():
    if "nc" not in _CACHE:
        _CACHE["nc"] = _build_nc_raw() if USE_RAW else _build_nc()
    return _CACHE["nc"]


def _host_prep(x, A, importance):
    """Returns in_maps (list of 8 dicts keyed by DRAM tensor name)."""
    x = np.asarray(x, dtype=np.float32)
    A = np.asarray(A, dtype=np.float32)
    importance = np.asarray(importance, dtype=np.float32)

    x0 = np.ascontiguousarray(x[:, 0, :, :])            # (N, T, V)

    a_full = A[None, :, :, :] * importance              # (O, K, V, V)
    B = (a_full[:, 0] + a_full[:, 1]) + a_full[:, 2]    # (O, V, V)
    bmat = B.transpose(1, 0, 2).reshape(V, OW)          # [v, o*V + w]
    bpad = np.zeros((96, NPAD), np.float32)
    for j in range(3):
        bpad[32 * j:32 * j + V, :OW] = bmat

    impt = np.ascontiguousarray(
        importance.transpose(2, 0, 1, 3).reshape(V, O * K * V))
    at8 = np.ascontiguousarray(
        np.tile(A.transpose(1, 0, 2).reshape(V, K * V), (1, O)))

    in_maps = []
    for c in range(NCORES):
        xt = x0[c * NLOC:(c + 1) * NLOC].reshape(TOK, V).T   # (V, TOK)
        xpk = np.zeros((96, XCOLS), np.float32)
        for j in range(3):
            ncols = 128 * GRP_NTILES[j]
            t0 = 128 * GRP_TOFF[j]
            xpk[32 * j:32 * j + V, :ncols] = xt[:, t0:t0 + ncols]
        in_maps.append({
            "xp": xpk,
            "bpad": bpad,
            "impt": impt,
            "at8": at8,
        })
    return in_maps


def _gather(results):
    y = np.empty((N, O, T, V), np.float32)
    for c in range(NCORES):
        yr = np.asarray(results[c]["y"])        # (128, 11, 3, OW) p-major
        yt = np.transpose(yr, (1, 2, 0, 3))     # (lt, j, p, OW)
        tok = np.empty((NTILE, 128, OW), np.float32)
        for j in range(3):
            nt = GRP_NTILES[j]
            tok[GRP_TOFF[j]:GRP_TOFF[j] + nt] = yt[:nt, j]
        yc = tok.reshape(NLOC, T, O, V)
        y[c * NLOC:(c + 1) * NLOC] = yc.transpose(0, 2, 1, 3)
    aft = np.asarray(results[0]["afullt"])               # (V, O*K*V)
    a_full = np.ascontiguousarray(
        aft.reshape(V, O, K, V).transpose(1, 2, 0, 3))   # (O, K, V, V)
    return y, a_full


def kernel(x, A, importance):
    from concourse.bass_utils import run_bass_kernel_spmd

    nc = _get_nc()
    in_maps = _host_prep(x, A, importance)
    res = run_bass_kernel_spmd(nc, in_maps, core_ids=list(range(NCORES)))
    _CACHE["last_results"] = res
    return _gather(res.results)


# revision 19
# speedup vs baseline: 1.1566x; 1.0149x over previous
"""Trainium2 Bass kernel for nn_ConvTemporalGraphical (gnn_message_passing).

Reference computation (see problem):
    A_full[o,k,v,w] = A[k,v,w] * importance[o,k,v,w]          (O,K,V,V)
    y[n,o,t,w]      = sum_{k,v} x[n,0,t,v] * A_full[o,k,v,w]  (N,O,T,V)
    returns (y, A_full)

Only channel 0 of x is used. The k-sum factors out of the x contraction:
    y[n,o,t,w] = sum_v x[n,0,t,v] * B[o,v,w],  B = sum_k A_full[o,k,:,:]
so the device work is a skinny matmul: (N*T, 25) @ (25, 200) per batch shard.

Sharding: data-parallel over batch N across 8 cores (8 batches/core ->
4096 tokens/core). B / importance are tiny and replicated.

Device layout per core:
  - x^T is packed on host into 3 row-groups at partition bases 0/32/64
    (the only legal engine AP bases), 11/11/10 token-tiles per group:
    xp[32*j + v, t'] = x^T[v, tok_off_j + t'].  DMAs are 96 partitions
    wide with multi-KB contiguous runs.
  - B is padded to (25, 256) free (moving free dim 256 => float32r
    streams 1 row/cycle) and replicated at partition bases 0/32/64.
  - 32 matmuls: lhsT = xp[32j:32j+25, 128-token slice] (stationary),
    rhs = bpad[32j:32j+25, :256] (moving), out = PSUM (128, 256) fp32.
  - PSUM -> SBUF copies (DVE/ACT split), staged 4 token-tiles per ybuf,
    8 output DMAs of (128, 4, 200).
  - A_full (an output) is computed on device as impT * at8 in
    [v, (o,k,w)] layout and DMA'd out; host restores the (O,K,V,V) order.
"""

import numpy as np

N, C, T, V = 64, 64, 512, 25
O, K = 8, 3
NCORES = 8
NLOC = N // NCORES          # 8 batches per core
TOK = NLOC * T              # 4096 tokens per core
OW = O * V                  # 200
NTILE = TOK // 128          # 32 token tiles

# 3 row-groups at partition bases 0/32/64: (ntiles) per group
GRP_NTILES = [11, 11, 10]
GRP_TOFF = [0, 11, 22]      # first global tile of each group
XCOLS = 128 * max(GRP_NTILES)      # 1408
XCHUNK_COLS = [512, 512, 384]      # column chunks (128-aligned)

# fp32 matmul is exact (4 cycles/moving-row); float32r is ~TF32 precision
# (~3e-4 rel err on this problem) but streams 1 cycle/row at N>=256.
import os as _os
USE_F32R = _os.environ.get("KBASS_F32R", "0") == "1"
NPAD = 256 if USE_F32R else OW

# raw-Bass pipeline (manual semaphores) avoids the Tile framework's
# preamble/tail barriers (~10us on a ~20us kernel)
USE_RAW = True

_CACHE = {}


def _tile_to_group(gt):
    for j in range(3):
        if gt < GRP_TOFF[j] + GRP_NTILES[j]:
            return j, gt - GRP_TOFF[j]
    raise ValueError(gt)


def _build_nc():
    import concourse.bacc as bacc
    import concourse.mybir as mybir
    import concourse.tile as tile

    f32 = mybir.dt.float32
    mmdt = mybir.dt.float32r if USE_F32R else f32

    nc = bacc.Bacc("TRN2", target_bir_lowering=False, debug=False,
                   enable_asserts=False)

    xp_d = nc.dram_tensor("xp", [96, XCOLS], mmdt, kind="ExternalInput")
    bpad_d = nc.dram_tensor("bpad", [96, NPAD], mmdt, kind="ExternalInput")
    impt_d = nc.dram_tensor("impt", [V, O * K * V], f32, kind="ExternalInput")
    at8_d = nc.dram_tensor("at8", [V, O * K * V], f32, kind="ExternalInput")
    # y is stored PARTITION-MAJOR: [p, round(lt), group(j), w-col]. Each
    # partition's data for a flush of consecutive rounds is contiguous in
    # DRAM, so a 2-round flush is 128 descriptors of 4800B (cheap HWDGE
    # issue + efficient SDMA runs). Host restores token-major order.
    y_d = nc.dram_tensor("y", [128, max(GRP_NTILES), 3, OW], f32,
                         kind="ExternalOutput")
    afullt_d = nc.dram_tensor("afullt", [V, O * K * V], f32, kind="ExternalOutput")

    with tile.TileContext(nc) as tc:
        with (
            tc.tile_pool(name="const", bufs=1) as cpool,
            tc.tile_pool(name="ybuf", bufs=6) as ypool,
            tc.tile_pool(name="psum", bufs=8, space="PSUM") as pspool,
        ):
            # B weights (host-prepped, replicated at bases 0/32/64):
            # issue on the ACT HWDGE ring, concurrent with x on the SP ring.
            bp = cpool.tile([96, NPAD], mmdt)
            nc.scalar.dma_start(bp[:], bpad_d[:])

            # x^T in 3 column chunks on the SP ring.
            xch = []
            cb = 0
            for cidx, cw in enumerate(XCHUNK_COLS):
                t = cpool.tile([96, cw], mmdt, tag=f"xch{cidx}")
                nc.sync.dma_start(t[:], xp_d[:, cb:cb + cw])
                xch.append((cb, t))
                cb += cw

            # Tiny A_full inputs on gpsimd (SWDGE) to keep HWDGE rings clear.
            im = cpool.tile([V, O * K * V], f32)
            nc.gpsimd.dma_start(im[:], impt_d[:])
            a8 = cpool.tile([V, O * K * V], f32)
            nc.gpsimd.dma_start(a8[:], at8_d[:])

            # Main loop, round-robin across the 3 row-groups so consecutive
            # matmuls land on independent 32-row PE tiles (they overlap).
            # Rounds are staged in pairs in ybuf and flushed with one
            # 128-descriptor DMA per pair on the SP ring.
            yb = None
            for lt in range(max(GRP_NTILES)):
                if lt % 2 == 0:
                    yb = ypool.tile([128, 2, 3, OW], f32, name="yb", tag="yb")
                for j in range(3):
                    if lt >= GRP_NTILES[j]:
                        continue
                    col = 128 * lt
                    chunk = min(col // 512, 2)
                    cb, xt = xch[chunk]
                    ps = pspool.tile([128, NPAD], f32, tag="ps")
                    lhsT = xt[32 * j:32 * j + V, col - cb:col - cb + 128]
                    rhs = bp[32 * j:32 * j + V, :]
                    nc.tensor.matmul(ps[:], lhsT, rhs)
                    if j < 2:
                        nc.vector.tensor_copy(yb[:, lt % 2, j, :], ps[:, 0:OW])
                    else:
                        nc.scalar.copy(yb[:, lt % 2, j, :], ps[:, 0:OW])
                if lt % 2 == 1:
                    nc.sync.dma_start(y_d[:, lt - 1:lt + 1, :, :], yb[:])
            # last round (lt=10) has groups 0,1 only
            nc.sync.dma_start(y_d[:, 10:11, 0:2, :], yb[:, 0:1, 0:2, :])

            # A_full output: prod[v, (o,k,w)] = impT * at8 — fully off the
            # critical path (DVE op + SWDGE store, emitted last).
            pr = cpool.tile([V, O * K * V], f32)
            nc.vector.tensor_mul(pr[:], im[:], a8[:])
            nc.gpsimd.dma_start(afullt_d[:], pr[:])

    nc.compile()
    return nc


def _mm_order():
    """MM index i -> (lt, j); round-robin j inner so consecutive matmuls hit
    independent 32-row PE tiles."""
    out = []
    for lt in range(max(GRP_NTILES)):
        for j in range(3):
            if lt < GRP_NTILES[j]:
                out.append((lt, j))
    return out


def _build_nc_raw():
    """Hand-scheduled pipeline: no TileContext, manual semaphores.

    Engine programs:
      sync   : x chunk loads, then one y flush per round-pair
      scalar : bpad load, then the j==2 PSUM->SBUF copies
      vector : the j<2 copies, then the A_full product
      tensor : 32 fp32 matmuls (waits: inputs, PSUM slot recycled)
      gpsimd : impT/at8 loads, A_full store
    """
    import contextlib

    import concourse.bass as bass
    import concourse.mybir as mybir

    f32 = mybir.dt.float32
    mmdt = mybir.dt.float32r if USE_F32R else f32

    nc = bass.Bass("TRN2", target_bir_lowering=False, debug=False,
                   enable_asserts=False)

    xp_d = nc.dram_tensor("xp", [96, XCOLS], mmdt, kind="ExternalInput")
    bpad_d = nc.dram_tensor("bpad", [96, NPAD], mmdt, kind="ExternalInput")
    impt_d = nc.dram_tensor("impt", [V, O * K * V], f32, kind="ExternalInput")
    at8_d = nc.dram_tensor("at8", [V, O * K * V], f32, kind="ExternalInput")
    y_d = nc.dram_tensor("y", [128, max(GRP_NTILES), 3, OW], f32,
                         kind="ExternalOutput")
    afullt_d = nc.dram_tensor("afullt", [V, O * K * V], f32, kind="ExternalOutput")

    order = _mm_order()
    nmm = len(order)                       # 32
    chunk_of = lambda lt: min(128 * lt // 512, 2)
    # copy engine per MM: DVE for j<2, ACT for j==2
    cp_eng = ["dve" if j < 2 else "act" for (lt, j) in order]
    # cumulative per-engine copy counts after MM i completes its copy
    dve_cum, act_cum = [], []
    dv = ac = 0
    for e in cp_eng:
        if e == "dve":
            dv += 1
        else:
            ac += 1
        dve_cum.append(dv)
        act_cum.append(ac)
    npairs = (max(GRP_NTILES) + 1) // 2    # 6 (last is the lone lt=10 round)
    # per-pair cumulative copy counts (y flush p waits for these)
    pair_last_mm = [max(i for i, (lt, j) in enumerate(order) if lt // 2 == p)
                    for p in range(npairs)]

    with contextlib.ExitStack() as ctx:
        sb = lambda shape, dt_, name: ctx.enter_context(
            nc.sbuf_tensor(name, shape, dt_))
        xch = [sb([96, XCHUNK_COLS[c]], mmdt, f"xch{c}") for c in range(3)]
        bp = sb([96, NPAD], mmdt, "bp")
        ybuf = [sb([128, 2 * 3 * OW], f32, f"ybuf{b}") for b in range(3)]
        im = sb([V, O * K * V], f32, "im")
        a8 = sb([V, O * K * V], f32, "a8")
        pr = sb([V, O * K * V], f32, "pr")
        psum = [ctx.enter_context(
            nc.psum_tensor(f"ps{s}", [128, 512], f32)) for s in range(8)]

        sem_x = [ctx.enter_context(nc.semaphore(f"sem_x{c}")) for c in range(3)]
        sem_b = ctx.enter_context(nc.semaphore("sem_b"))
        sem_g = ctx.enter_context(nc.semaphore("sem_g"))
        sem_yb = [ctx.enter_context(nc.semaphore(f"sem_yb{b}")) for b in range(3)]
        mm_sem = ctx.enter_context(nc.semaphore("mm_sem"))
        cp_dve = ctx.enter_context(nc.semaphore("cp_dve"))
        cp_act = ctx.enter_context(nc.semaphore("cp_act"))
        mul_sem = ctx.enter_context(nc.semaphore("mul_sem"))

        block = ctx.enter_context(nc.Block())

        def emit_flush(eng, p):
            i_last = pair_last_mm[p]
            eng.wait_ge(cp_dve, dve_cum[i_last])
            if act_cum[i_last]:
                eng.wait_ge(cp_act, act_cum[i_last])
            if p < npairs - 1:
                eng.dma_start(
                    y_d[:, 2 * p:2 * p + 2, :, :],
                    ybuf[p % 3][:].rearrange(
                        "p (r j c) -> p r j c", r=2, j=3),
                ).then_inc(sem_yb[p % 3], 16)
            else:  # lone lt=10 round: groups 0,1 only
                eng.dma_start(
                    y_d[:, 10:11, 0:2, :],
                    ybuf[p % 3][:, 0:2 * OW].rearrange(
                        "p (r j c) -> p r j c", r=1, j=2),
                ).then_inc(sem_yb[p % 3], 16)

        @block.sync
        def _(sync):
            # bpad first: it gates the first matmul and this ring spins up
            # earliest; chunk0 right behind it.
            sync.dma_start(bp[:], bpad_d[:]).then_inc(sem_b, 16)
            sync.dma_start(xch[0][:], xp_d[:, 0:512]).then_inc(sem_x[0], 16)
            for p in range(0, npairs, 2):
                emit_flush(sync, p)
            for b in range(3):
                sync.wait_ge(sem_yb[b], 32)

        @block.tensor
        def _(tensor):
            emitted_x = 0
            for i, (lt, j) in enumerate(order):
                ch = chunk_of(lt)
                if i == 0:
                    tensor.wait_ge(sem_b, 16)
                if ch + 1 > emitted_x:
                    tensor.wait_ge(sem_x[ch], 16)
                    emitted_x = ch + 1
                if i >= 8:
                    ip = i - 8
                    if cp_eng[ip] == "dve":
                        tensor.wait_ge(cp_dve, dve_cum[ip])
                    else:
                        tensor.wait_ge(cp_act, act_cum[ip])
                col = 128 * lt
                cb = [0, 512, 1024][ch]
                tensor.matmul(
                    psum[i % 8][:, 0:NPAD],
                    xch[ch][32 * j:32 * j + V, col - cb:col - cb + 128],
                    bp[32 * j:32 * j + V, :],
                ).then_inc(mm_sem)

        @block.vector
        def _(vector):
            seen_pair = -1
            for i, (lt, j) in enumerate(order):
                if cp_eng[i] != "dve":
                    continue
                p = lt // 2
                if p >= 3 and p != seen_pair:
                    vector.wait_ge(sem_yb[p % 3], 16)
                seen_pair = max(seen_pair, p)
                vector.wait_ge(mm_sem, i + 1)
                vector.tensor_copy(
                    ybuf[p % 3][:, ((lt % 2) * 3 + j) * OW:
                                ((lt % 2) * 3 + j + 1) * OW],
                    psum[i % 8][:, 0:OW],
                ).then_inc(cp_dve)
            vector.wait_ge(sem_g, 32)
            vector.tensor_mul(pr[:], im[:], a8[:]).then_inc(mul_sem)

        @block.scalar
        def _(scalar):
            scalar.dma_start(xch[1][:], xp_d[:, 512:1024]).then_inc(sem_x[1], 16)
            scalar.dma_start(xch[2][:], xp_d[:, 1024:1408]).then_inc(sem_x[2], 16)
            seen_pair = -1
            flushed = set()
            for i, (lt, j) in enumerate(order):
                if cp_eng[i] != "act":
                    continue
                p = lt // 2
                # odd-pair flushes interleave with this engine's copies:
                # flush pair q as soon as its last copy (always an earlier
                # instruction in this stream or DVE's) is done
                for q in range(1, npairs, 2):
                    if q not in flushed and pair_last_mm[q] < i:
                        emit_flush(scalar, q)
                        flushed.add(q)
                if p >= 3 and p != seen_pair:
                    scalar.wait_ge(sem_yb[p % 3], 16)
                seen_pair = max(seen_pair, p)
                scalar.wait_ge(mm_sem, i + 1)
                scalar.copy(
                    ybuf[p % 3][:, ((lt % 2) * 3 + j) * OW:
                                ((lt % 2) * 3 + j + 1) * OW],
                    psum[i % 8][:, 0:OW],
                ).then_inc(cp_act)
            for q in range(1, npairs, 2):
                if q not in flushed:
                    emit_flush(scalar, q)
            for b in range(3):
                scalar.wait_ge(sem_yb[b], 32)

        @block.gpsimd
        def _(gpsimd):
            gpsimd.dma_start(im[:], impt_d[:]).then_inc(sem_g, 16)
            gpsimd.dma_start(a8[:], at8_d[:]).then_inc(sem_g, 16)
            gpsimd.wait_ge(mul_sem, 1)
            gpsimd.dma_start(afullt_d[:], pr[:]).then_inc(sem_g, 16)
            gpsimd.wait_ge(sem_g, 48)

    return nc


def _get_nc():
    if "nc" not in _CACHE:
        _CACHE["nc"] = _build_nc_raw() if USE_RAW else _build_nc()
    return _CACHE["nc"]


def _host_prep(x, A, importance):
    """Returns in_maps (list of 8 dicts keyed by DRAM tensor name)."""
    x = np.asarray(x, dtype=np.float32)
    A = np.asarray(A, dtype=np.float32)
    importance = np.asarray(importance, dtype=np.float32)

    x0 = np.ascontiguousarray(x[:, 0, :, :])            # (N, T, V)

    a_full = A[None, :, :, :] * importance              # (O, K, V, V)
    B = (a_full[:, 0] + a_full[:, 1]) + a_full[:, 2]    # (O, V, V)
    bmat = B.transpose(1, 0, 2).reshape(V, OW)          # [v, o*V + w]
    bpad = np.zeros((96, NPAD), np.float32)
    for j in range(3):
        bpad[32 * j:32 * j + V, :OW] = bmat

    impt = np.ascontiguousarray(
        importance.transpose(2, 0, 1, 3).reshape(V, O * K * V))
    at8 = np.ascontiguousarray(
        np.tile(A.transpose(1, 0, 2).reshape(V, K * V), (1, O)))

    in_maps = []
    for c in range(NCORES):
        xt = x0[c * NLOC:(c + 1) * NLOC].reshape(TOK, V).T   # (V, TOK)
        xpk = np.zeros((96, XCOLS), np.float32)
        for j in range(3):
            ncols = 128 * GRP_NTILES[j]
            t0 = 128 * GRP_TOFF[j]
            xpk[32 * j:32 * j + V, :ncols] = xt[:, t0:t0 + ncols]
        in_maps.append({
            "xp": xpk,
            "bpad": bpad,
            "impt": impt,
            "at8": at8,
        })
    return in_maps


def _gather(results):
    y = np.empty((N, O, T, V), np.float32)
    for c in range(NCORES):
        yr = np.asarray(results[c]["y"])        # (128, 11, 3, OW) p-major
        yt = np.transpose(yr, (1, 2, 0, 3))     # (lt, j, p, OW)
        tok = np.empty((NTILE, 128, OW), np.float32)
        for j in range(3):
            nt = GRP_NTILES[j]
            tok[GRP_TOFF[j]:GRP_TOFF[j] + nt] = yt[:nt, j]
        yc = tok.reshape(NLOC, T, O, V)
        y[c * NLOC:(c + 1) * NLOC] = yc.transpose(0, 2, 1, 3)
    aft = np.asarray(results[0]["afullt"])               # (V, O*K*V)
    a_full = np.ascontiguousarray(
        aft.reshape(V, O, K, V).transpose(1, 2, 0, 3))   # (O, K, V, V)
    return y, a_full


def kernel(x, A, importance):
    from concourse.bass_utils import run_bass_kernel_spmd

    nc = _get_nc()
    in_maps = _host_prep(x, A, importance)
    res = run_bass_kernel_spmd(nc, in_maps, core_ids=list(range(NCORES)))
    _CACHE["last_results"] = res
    return _gather(res.results)
